# revision 4
# baseline (speedup 1.0000x reference)
"""CKA loss kernel for Trainium2 (8 NeuronCores, SPMD).

Math: for X_i = output[i] (shape [N, D]), the reference builds N x N Gram
matrices K_i = X_i X_i^T (diag zeroed), double-centers them and takes
pairwise inner products. Algebraically this reduces to D x D cross-Grams:

    S[i,j] = <Khat_i, Khat_j>
           = ||X_i^T X_j||_F^2 - g_i.g_j - (2/N) r_i.r_j + t_i t_j / N^2

with g_i[n] = ||X_i[n]||^2, r_i = X_i (X_i^T 1) - g_i, t_i = sum(r_i).
Only the ||X_i^T X_j||_F^2 term is heavy (C_ij = X_i^T X_j, 512x512,
contraction over N=4096) — everything else is O(M N D) and done on host
in float64.

Sharding: the contraction axis N is split across the 8 cores (512 samples
each). Every core computes partial C_ij for all 21 pairs (i<=j) from its
slice — perfectly balanced, no collectives. The host sums the partials in
float64, squares, and assembles the tiny [6,6] output.

For diagonal pairs C_ii only column-blocks e >= d-block are computed
(block-level symmetry), saving ~12% of the matmul work.
"""

import numpy as np

M, N, D = 6, 4096, 512
NCORES = 8
NLOC = N // NCORES   # samples per core
P = 128              # partitions
NCH = NLOC // P      # contraction chunks per core
KBLK = D // P        # 128-row blocks of C
PAIRS = [(i, j) for i in range(M) for j in range(i, M)]  # 21
EPS = 1e-6

_CACHE = {}


def _build_nc(mm_dtype_name="float32r"):
    import concourse.bacc as bacc
    import concourse.mybir as mybir
    import concourse.tile as tile

    mm_dt = getattr(mybir.dt, mm_dtype_name)
    nc = bacc.Bacc("TRN2", target_bir_lowering=False, debug=False,
                   num_devices=NCORES)
    x_in = nc.dram_tensor("x", [M, NLOC, D], mybir.dt.float32,
                          kind="ExternalInput")
    cout = nc.dram_tensor("cout", [len(PAIRS), KBLK, P, D], mybir.dt.float32,
                          kind="ExternalOutput")

    with tile.TileContext(nc) as tc:
        with (
            tc.tile_pool(name="xin", bufs=1) as xpool,
            tc.tile_pool(name="stage", bufs=4) as spool,
            tc.tile_pool(name="psum", bufs=8, space="PSUM") as ppool,
        ):
            xs = []
            for i in range(M):
                xt = xpool.tile([P, NCH, D], mybir.dt.float32, tag=f"x{i}")
                nc.sync.dma_start(
                    xt[:], x_in[i].rearrange("(c p) d -> p c d", p=P)
                )
                if mm_dt == mybir.dt.float32:
                    xs.append(xt)
                else:
                    # fp32r operands must be produced by an op that rounds
                    # to fp32r (BIR verifier requirement)
                    xr = xpool.tile([P, NCH, D], mm_dt, tag=f"xr{i}")
                    nc.vector.tensor_copy(xr[:], xt[:])
                    xs.append(xr)

            for pi, (i, j) in enumerate(PAIRS):
                for k in range(KBLK):
                    # C_ij rows [128k, 128k+128); for i==j skip column
                    # blocks left of the diagonal block.
                    e0 = k * P if i == j else 0
                    w = D - e0
                    acc = ppool.tile([P, D], mybir.dt.float32, tag="acc")
                    for c in range(NCH):
                        nc.tensor.matmul(
                            acc[:, :w],
                            xs[i][:, c, k * P:(k + 1) * P],
                            xs[j][:, c, e0:],
                            start=(c == 0),
                            stop=(c == NCH - 1),
                        )
                    st = spool.tile([P, D], mybir.dt.float32, tag="st")
                    nc.vector.tensor_copy(st[:, :w], acc[:, :w])
                    nc.sync.dma_start(cout[pi, k, :, :w], st[:, :w])

    nc.compile()
    return nc


def _get_nc():
    if "nc" not in _CACHE:
        _CACHE["nc"] = _build_nc()
    return _CACHE["nc"]


def _run_device(X, trace=False):
    """X: [M, N, D] float32 -> per-core partial C blocks, list of
    [21, KBLK, P, D] float32 arrays (and the raw results object)."""
    from concourse.bass_utils import run_bass_kernel_spmd

    nc = _get_nc()
    in_maps = [
        {"x": np.ascontiguousarray(X[:, c * NLOC:(c + 1) * NLOC, :])}
        for c in range(NCORES)
    ]
    res = run_bass_kernel_spmd(nc, in_maps, core_ids=list(range(NCORES)),
                               trace=trace)
    return res


def _assemble(X, couts):
    """Host-side float64 assembly of the final CKA outputs."""
    Csum = np.zeros((len(PAIRS), KBLK, P, D), np.float64)
    for c in couts:
        Csum += c.astype(np.float64)

    F2 = np.zeros((M, M))
    for pi, (i, j) in enumerate(PAIRS):
        if i != j:
            v = np.sum(Csum[pi] ** 2)
        else:
            v = 0.0
            for k in range(KBLK):
                # block row k holds column blocks l >= k, packed at col 0:
                # local col c maps to global e = 128k + c, valid width w
                w = D - k * P
                row = Csum[pi, k]  # [P, D]; valid cols are :w
                v += np.sum(row[:, :P] ** 2)          # diagonal block (k,k)
                if w > P:
                    v += 2.0 * np.sum(row[:, P:w] ** 2)  # blocks l > k
        F2[i, j] = v
        F2[j, i] = v

    Xd = X.astype(np.float64)
    g = np.einsum("ind,ind->in", Xd, Xd)            # [M, N]
    s = Xd.sum(axis=1)                              # [M, D]
    r = np.einsum("ind,id->in", Xd, s) - g          # [M, N]
    t = r.sum(axis=1)                               # [M]

    S = F2 - g @ g.T - (2.0 / N) * (r @ r.T) + np.outer(t, t) / N ** 2
    S = np.abs(S)
    diag = np.diagonal(S)
    cka = S / np.sqrt(diag[:, None] * diag[None, :] + EPS)
    hsic = cka * np.tril(np.ones((M, M)), k=-1)
    l = np.sum(np.abs(hsic))
    vis = hsic + hsic.T + np.eye(M)
    return vis.astype(np.float32), np.array(l, dtype=np.float32)


def kernel(output):
    X = np.ascontiguousarray(np.asarray(output), dtype=np.float32)
    assert X.shape == (M, N, D)
    res = _run_device(X)
    couts = [r["cout"] for r in res.results]
    return _assemble(X, couts)


# revision 6
# speedup vs baseline: 1.2331x; 1.2331x over previous
"""CKA loss kernel for Trainium2 (8 NeuronCores, SPMD).

Math: for X_i = output[i] (shape [N, D]), the reference builds N x N Gram
matrices K_i = X_i X_i^T (diag zeroed), double-centers them and takes
pairwise inner products. Algebraically this reduces to D x D cross-Grams:

    S[i,j] = <Khat_i, Khat_j>
           = ||X_i^T X_j||_F^2 - g_i.g_j - (2/N) r_i.r_j + t_i t_j / N^2

with g_i[n] = ||X_i[n]||^2, r_i = X_i (X_i^T 1) - g_i, t_i = sum(r_i).
Only the ||X_i^T X_j||_F^2 term is heavy (C_ij = X_i^T X_j, 512x512,
contraction over N=4096) — everything else is O(M N D) and done on host
in float64.

Sharding: the contraction axis N is split across the 8 cores (512 samples
each). Every core computes partial C_ij for all 21 pairs (i<=j) from its
slice — perfectly balanced, no collectives. The host sums the partials in
float64, squares, and assembles the tiny [6,6] output.

The matmuls run in float16 (inputs are ~N(0,1); fp16 rounding of the
inputs perturbs S by ~1e-5 relative — the final cancellation is only ~8x).
Partial C blocks return to the host as float16 as well (entries are
O(100), and the per-entry rounding is ~2^-11 relative, far below the
tolerance). For diagonal pairs C_ii only column-blocks e >= d-block are
computed (block-level symmetry), saving ~12% of the matmul work.
"""

import numpy as np

M, N, D = 6, 4096, 512
NCORES = 8
NLOC = N // NCORES   # samples per core
P = 128              # partitions
NCH = NLOC // P      # contraction chunks per core
KBLK = D // P        # 128-row blocks of C
PAIRS = [(i, j) for i in range(M) for j in range(i, M)]  # 21
EPS = 1e-6

_CACHE = {}


def _build_nc(mm_dtype_name="float16"):
    import concourse.bacc as bacc
    import concourse.mybir as mybir
    import concourse.tile as tile

    mm_dt = getattr(mybir.dt, mm_dtype_name)
    nc = bacc.Bacc("TRN2", target_bir_lowering=False, debug=False,
                   num_devices=NCORES)
    # host supplies [M, P, NCH, D]: partition-major so each partition's
    # line is one contiguous 4KB run for the DMA
    x_in = nc.dram_tensor("x", [M, P, NCH, D], mm_dt, kind="ExternalInput")
    cout = nc.dram_tensor("cout", [len(PAIRS), P, KBLK * D], mm_dt,
                          kind="ExternalOutput")

    with tile.TileContext(nc) as tc:
        with (
            tc.tile_pool(name="xin", bufs=1) as xpool,
            tc.tile_pool(name="stage", bufs=3) as spool,
            tc.tile_pool(name="psum", bufs=8, space="PSUM") as ppool,
        ):
            xs = []
            for i in range(M):
                xt = xpool.tile([P, NCH, D], mm_dt, tag=f"x{i}")
                nc.sync.dma_start(xt[:], x_in[i])
                xs.append(xt)

            ncopy = 0
            for pi, (i, j) in enumerate(PAIRS):
                st = spool.tile([P, KBLK * D], mm_dt, tag="st")
                for k in range(KBLK):
                    # C_ij rows [128k, 128k+128); for i==j skip column
                    # blocks left of the diagonal block.
                    e0 = k * P if i == j else 0
                    w = D - e0
                    acc = ppool.tile([P, D], mybir.dt.float32, tag="acc")
                    for c in range(NCH):
                        nc.tensor.matmul(
                            acc[:, :w],
                            xs[i][:, c, k * P:(k + 1) * P],
                            xs[j][:, c, e0:],
                            start=(c == 0),
                            stop=(c == NCH - 1),
                        )
                    # spread PSUM->SBUF cast-copies over DVE and ACT
                    dst = st[:, k * D:k * D + w]
                    if ncopy % 3 == 2:
                        nc.scalar.copy(dst, acc[:, :w])
                    else:
                        nc.vector.tensor_copy(dst, acc[:, :w])
                    ncopy += 1
                nc.sync.dma_start(cout[pi], st[:])

    nc.compile()
    return nc


def _get_nc():
    if "nc" not in _CACHE:
        _CACHE["nc"] = _build_nc()
    return _CACHE["nc"]


def _shard_inputs(X):
    """X: [M, N, D] float32 -> per-core float16 [M, P, NCH, D] arrays."""
    Xh = X.astype(np.float16)
    maps = []
    for c in range(NCORES):
        sl = Xh[:, c * NLOC:(c + 1) * NLOC, :]
        sl = sl.reshape(M, NCH, P, D).transpose(0, 2, 1, 3)
        maps.append({"x": np.ascontiguousarray(sl)})
    return maps


def _run_device(X, trace=False):
    from concourse.bass_utils import run_bass_kernel_spmd

    nc = _get_nc()
    res = run_bass_kernel_spmd(nc, _shard_inputs(X),
                               core_ids=list(range(NCORES)), trace=trace)
    return res


def _assemble(X, couts):
    """Host-side float64 assembly of the final CKA outputs.

    couts: per-core [21, P, KBLK*D] arrays of partial C blocks; block k
    of pair (i,j) occupies columns [k*D, k*D + w) where w = D - 128k for
    diagonal pairs (packed at 0) and w = D otherwise.
    """
    Csum = np.zeros((len(PAIRS), P, KBLK * D), np.float64)
    for c in couts:
        Csum += c.astype(np.float64)

    F2 = np.zeros((M, M))
    for pi, (i, j) in enumerate(PAIRS):
        if i != j:
            v = np.sum(Csum[pi] ** 2)
        else:
            v = 0.0
            for k in range(KBLK):
                w = D - k * P
                row = Csum[pi, :, k * D:k * D + w]
                v += np.sum(row[:, :P] ** 2)            # diagonal block (k,k)
                if w > P:
                    v += 2.0 * np.sum(row[:, P:] ** 2)  # blocks l > k
        F2[i, j] = v
        F2[j, i] = v

    # use the same fp16-rounded X the device matmuls consumed, so the
    # g/r/t terms cancel the diagonal contributions of ||C||^2 exactly
    Xd = X.astype(np.float16).astype(np.float64)
    g = np.einsum("ind,ind->in", Xd, Xd)            # [M, N]
    s = Xd.sum(axis=1)                              # [M, D]
    r = np.einsum("ind,id->in", Xd, s) - g          # [M, N]
    t = r.sum(axis=1)                               # [M]

    S = F2 - g @ g.T - (2.0 / N) * (r @ r.T) + np.outer(t, t) / N ** 2
    S = np.abs(S)
    diag = np.diagonal(S)
    cka = S / np.sqrt(diag[:, None] * diag[None, :] + EPS)
    hsic = cka * np.tril(np.ones((M, M)), k=-1)
    l = np.sum(np.abs(hsic))
    vis = hsic + hsic.T + np.eye(M)
    return vis.astype(np.float32), np.array(l, dtype=np.float32)


def kernel(output):
    X = np.ascontiguousarray(np.asarray(output), dtype=np.float32)
    assert X.shape == (M, N, D)
    res = _run_device(X)
    couts = [r["cout"] for r in res.results]
    return _assemble(X, couts)


# revision 11
# speedup vs baseline: 1.2489x; 1.0128x over previous
"""CKA loss kernel for Trainium2 (8 NeuronCores, SPMD).

Math: for X_i = output[i] (shape [N, D]), the reference builds N x N Gram
matrices K_i = X_i X_i^T (diag zeroed), double-centers them and takes
pairwise inner products. Algebraically this reduces to D x D cross-Grams:

    S[i,j] = <Khat_i, Khat_j>
           = ||X_i^T X_j||_F^2 - g_i.g_j - (2/N) r_i.r_j + t_i t_j / N^2

with g_i[n] = ||X_i[n]||^2, r_i = X_i (X_i^T 1) - g_i, t_i = sum(r_i).
Only the ||X_i^T X_j||_F^2 term is heavy (C_ij = X_i^T X_j, 512x512,
contraction over N=4096) — everything else is O(M N D) and done on host
in float64.

Sharding: the contraction axis N is split across the 8 cores (512 samples
each). Every core computes partial C_ij for all 21 pairs (i<=j) from its
slice — perfectly balanced, no collectives. The host sums the partials in
float64, squares, and assembles the tiny [6,6] output.

The matmuls run in float16 (inputs are ~N(0,1); fp16 rounding of the
inputs perturbs S by ~1e-5 relative — the final cancellation is only ~8x).
Partial C blocks return to the host as float16 as well (entries are
O(100), and the per-entry rounding is ~2^-11 relative, far below the
tolerance). For diagonal pairs C_ii only column-blocks e >= d-block are
computed (block-level symmetry), saving ~12% of the matmul work.
"""

import numpy as np

M, N, D = 6, 4096, 512
NCORES = 8
NLOC = N // NCORES   # samples per core
P = 128              # partitions
NCH = NLOC // P      # contraction chunks per core
KBLK = D // P        # 128-row blocks of C
PAIRS = [(i, j) for i in range(M) for j in range(i, M)]  # 21
EPS = 1e-6

MM_DTYPE = "float16"  # matmul operand/output-staging dtype

_CACHE = {}


def _np_mm_dtype():
    if MM_DTYPE == "float16":
        return np.float16
    import ml_dtypes

    return np.dtype(ml_dtypes.bfloat16)


def _build_nc(mm_dtype_name=None):
    mm_dtype_name = mm_dtype_name or MM_DTYPE
    import concourse.bacc as bacc
    import concourse.mybir as mybir
    import concourse.tile as tile

    mm_dt = getattr(mybir.dt, mm_dtype_name)
    nc = bacc.Bacc("TRN2", target_bir_lowering=False, debug=False,
                   num_devices=NCORES)
    # host supplies [M, P, NCH, D]: partition-major so each partition's
    # line is one contiguous 4KB run for the DMA
    x_in = nc.dram_tensor("x", [M, P, NCH, D], mm_dt, kind="ExternalInput")
    cout = nc.dram_tensor("cout", [len(PAIRS), P, KBLK * D], mm_dt,
                          kind="ExternalOutput")

    with tile.TileContext(nc) as tc:
        with (
            tc.tile_pool(name="xin", bufs=1) as xpool,
            tc.tile_pool(name="stage", bufs=3) as spool,
            tc.tile_pool(name="psum", bufs=8, space="PSUM") as ppool,
        ):
            # one tile per (layer, chunk) so matmuls wait only on the
            # chunks they actually read — faster pipeline ramp
            xs = []
            for i in range(M):
                row = []
                for c in range(NCH):
                    xt = xpool.tile([P, D], mm_dt, tag=f"x{i}c{c}")
                    nc.sync.dma_start(xt[:], x_in[i, :, c, :])
                    row.append(xt)
                xs.append(row)

            ncopy = 0
            for pi, (i, j) in enumerate(PAIRS):
                st = spool.tile([P, KBLK * D], mm_dt, tag="st")
                for k in range(KBLK):
                    # C_ij rows [128k, 128k+128); for i==j skip column
                    # blocks left of the diagonal block.
                    e0 = k * P if i == j else 0
                    w = D - e0
                    acc = ppool.tile([P, D], mybir.dt.float32, tag="acc")
                    for c in range(NCH):
                        nc.tensor.matmul(
                            acc[:, :w],
                            xs[i][c][:, k * P:(k + 1) * P],
                            xs[j][c][:, e0:],
                            start=(c == 0),
                            stop=(c == NCH - 1),
                        )
                    # spread PSUM->SBUF cast-copies over DVE and ACT
                    dst = st[:, k * D:k * D + w]
                    if ncopy % 3 == 2:
                        nc.scalar.copy(dst, acc[:, :w])
                    else:
                        nc.vector.tensor_copy(dst, acc[:, :w])
                    ncopy += 1
                nc.sync.dma_start(cout[pi], st[:])

    nc.compile()
    return nc


def _get_nc():
    if "nc" not in _CACHE:
        _CACHE["nc"] = _build_nc()
    return _CACHE["nc"]


def _shard_inputs(X):
    """X: [M, N, D] float32 -> per-core low-precision [M, P, NCH, D]."""
    Xh = X.astype(_np_mm_dtype())
    maps = []
    for c in range(NCORES):
        sl = Xh[:, c * NLOC:(c + 1) * NLOC, :]
        sl = sl.reshape(M, NCH, P, D).transpose(0, 2, 1, 3)
        maps.append({"x": np.ascontiguousarray(sl)})
    return maps


def _run_device(X, trace=False):
    from concourse.bass_utils import run_bass_kernel_spmd

    nc = _get_nc()
    res = run_bass_kernel_spmd(nc, _shard_inputs(X),
                               core_ids=list(range(NCORES)), trace=trace)
    return res


def _assemble(X, couts):
    """Host-side float64 assembly of the final CKA outputs.

    couts: per-core [21, P, KBLK*D] arrays of partial C blocks; block k
    of pair (i,j) occupies columns [k*D, k*D + w) where w = D - 128k for
    diagonal pairs (packed at 0) and w = D otherwise.
    """
    Csum = np.zeros((len(PAIRS), P, KBLK * D), np.float64)
    for c in couts:
        Csum += c.astype(np.float64)

    F2 = np.zeros((M, M))
    for pi, (i, j) in enumerate(PAIRS):
        if i != j:
            v = np.sum(Csum[pi] ** 2)
        else:
            v = 0.0
            for k in range(KBLK):
                w = D - k * P
                row = Csum[pi, :, k * D:k * D + w]
                v += np.sum(row[:, :P] ** 2)            # diagonal block (k,k)
                if w > P:
                    v += 2.0 * np.sum(row[:, P:] ** 2)  # blocks l > k
        F2[i, j] = v
        F2[j, i] = v

    # use the same rounded X the device matmuls consumed, so the
    # g/r/t terms cancel the diagonal contributions of ||C||^2 exactly
    Xd = X.astype(_np_mm_dtype()).astype(np.float64)
    g = np.einsum("ind,ind->in", Xd, Xd)            # [M, N]
    s = Xd.sum(axis=1)                              # [M, D]
    r = np.einsum("ind,id->in", Xd, s) - g          # [M, N]
    t = r.sum(axis=1)                               # [M]

    S = F2 - g @ g.T - (2.0 / N) * (r @ r.T) + np.outer(t, t) / N ** 2
    S = np.abs(S)
    diag = np.diagonal(S)
    cka = S / np.sqrt(diag[:, None] * diag[None, :] + EPS)
    hsic = cka * np.tril(np.ones((M, M)), k=-1)
    l = np.sum(np.abs(hsic))
    vis = hsic + hsic.T + np.eye(M)
    return vis.astype(np.float32), np.array(l, dtype=np.float32)


def kernel(output):
    X = np.ascontiguousarray(np.asarray(output), dtype=np.float32)
    assert X.shape == (M, N, D)
    res = _run_device(X)
    couts = [r["cout"] for r in res.results]
    return _assemble(X, couts)


# revision 14
# speedup vs baseline: 1.2583x; 1.0075x over previous
"""CKA loss kernel for Trainium2 (8 NeuronCores, SPMD).

Math: for X_i = output[i] (shape [N, D]), the reference builds N x N Gram
matrices K_i = X_i X_i^T (diag zeroed), double-centers them and takes
pairwise inner products. Algebraically this reduces to D x D cross-Grams:

    S[i,j] = <Khat_i, Khat_j>
           = ||X_i^T X_j||_F^2 - g_i.g_j - (2/N) r_i.r_j + t_i t_j / N^2

with g_i[n] = ||X_i[n]||^2, r_i = X_i (X_i^T 1) - g_i, t_i = sum(r_i).
Only the ||X_i^T X_j||_F^2 term is heavy (C_ij = X_i^T X_j, 512x512,
contraction over N=4096) — everything else is O(M N D) and done on host
in float64.

Sharding: the contraction axis N is split across the 8 cores (512 samples
each). Every core computes partial C_ij for all 21 pairs (i<=j) from its
slice — perfectly balanced, no collectives. The host sums the partials in
float64, squares, and assembles the tiny [6,6] output.

The matmuls run in float16 (inputs are ~N(0,1); fp16 rounding of the
inputs perturbs S by ~1e-5 relative — the final cancellation is only ~8x).
Partial C blocks return to the host as float16 as well (entries are
O(100), and the per-entry rounding is ~2^-11 relative, far below the
tolerance). For diagonal pairs C_ii only column-blocks e >= d-block are
computed (block-level symmetry), saving ~12% of the matmul work.
"""

import numpy as np

M, N, D = 6, 4096, 512
NCORES = 8
NLOC = N // NCORES   # samples per core
P = 128              # partitions
NCH = NLOC // P      # contraction chunks per core
KBLK = D // P        # 128-row blocks of C
PAIRS = [(i, j) for i in range(M) for j in range(i, M)]  # 21
EPS = 1e-6

MM_DTYPE = "float16"  # matmul operand/output-staging dtype

_CACHE = {}


def _np_mm_dtype():
    if MM_DTYPE == "float16":
        return np.float16
    import ml_dtypes

    return np.dtype(ml_dtypes.bfloat16)


def _build_nc(mm_dtype_name=None):
    mm_dtype_name = mm_dtype_name or MM_DTYPE
    import concourse.bacc as bacc
    import concourse.mybir as mybir
    import concourse.tile as tile

    mm_dt = getattr(mybir.dt, mm_dtype_name)
    nc = bacc.Bacc("TRN2", target_bir_lowering=False, debug=False,
                   num_devices=NCORES)
    # chunk-major input: each (layer, chunk) slice is one contiguous
    # 128KB region -> large DMA descriptors
    x_in = nc.dram_tensor("x", [M, NCH, P, D], mm_dt, kind="ExternalInput")
    cout = nc.dram_tensor("cout", [len(PAIRS), P, KBLK * D], mm_dt,
                          kind="ExternalOutput")

    with tile.TileContext(nc) as tc:
        with (
            tc.tile_pool(name="xin", bufs=1) as xpool,
            tc.tile_pool(name="stage", bufs=3) as spool,
            tc.tile_pool(name="psum", bufs=8, space="PSUM") as ppool,
        ):
            # one tile per (layer, chunk) so matmuls wait only on the
            # chunks they actually read — faster pipeline ramp.  Input
            # DMAs trigger from GpSimd so the Sync queue is free for
            # output DMAs.
            xs = []
            for i in range(M):
                row = []
                for c in range(NCH):
                    xt = xpool.tile([P, D], mm_dt, tag=f"x{i}c{c}")
                    nc.gpsimd.dma_start(xt[:], x_in[i, c])
                    row.append(xt)
                xs.append(row)

            ncopy = 0
            for pi, (i, j) in enumerate(PAIRS):
                st = spool.tile([P, KBLK * D], mm_dt, tag="st")
                off = 0
                for k in range(KBLK):
                    # C_ij rows [128k, 128k+128); for i==j skip column
                    # blocks left of the diagonal block.
                    e0 = k * P if i == j else 0
                    w = D - e0
                    acc = ppool.tile([P, D], mybir.dt.float32, tag="acc")
                    for c in range(NCH):
                        nc.tensor.matmul(
                            acc[:, :w],
                            xs[i][c][:, k * P:(k + 1) * P],
                            xs[j][c][:, e0:],
                            start=(c == 0),
                            stop=(c == NCH - 1),
                        )
                    # spread PSUM->SBUF cast-copies over DVE and ACT
                    dst = st[:, off:off + w]
                    if ncopy % 3 == 2:
                        nc.scalar.copy(dst, acc[:, :w])
                    else:
                        nc.vector.tensor_copy(dst, acc[:, :w])
                    ncopy += 1
                    off += w
                nc.sync.dma_start(cout[pi, :, :off], st[:, :off])

    nc.compile()
    return nc


def _get_nc():
    if "nc" not in _CACHE:
        _CACHE["nc"] = _build_nc()
    return _CACHE["nc"]


def _shard_inputs(X):
    """X: [M, N, D] float32 -> per-core low-precision [M, NCH, P, D]."""
    Xh = X.astype(_np_mm_dtype())
    maps = []
    for c in range(NCORES):
        sl = Xh[:, c * NLOC:(c + 1) * NLOC, :].reshape(M, NCH, P, D)
        maps.append({"x": np.ascontiguousarray(sl)})
    return maps


def _run_device(X, trace=False):
    from concourse.bass_utils import run_bass_kernel_spmd

    nc = _get_nc()
    res = run_bass_kernel_spmd(nc, _shard_inputs(X),
                               core_ids=list(range(NCORES)), trace=trace)
    return res


def _assemble(X, couts):
    """Host-side float64 assembly of the final CKA outputs.

    couts: per-core [21, P, KBLK*D] arrays of partial C blocks; block k
    of pair (i,j) has width w_k (= D, or D - 128k for diagonal pairs)
    and the blocks are packed back-to-back from column 0.
    """
    Csum = np.zeros((len(PAIRS), P, KBLK * D), np.float64)
    for c in couts:
        Csum += c.astype(np.float64)

    F2 = np.zeros((M, M))
    for pi, (i, j) in enumerate(PAIRS):
        if i != j:
            v = np.sum(Csum[pi] ** 2)
        else:
            v = 0.0
            off = 0
            for k in range(KBLK):
                w = D - k * P
                row = Csum[pi, :, off:off + w]
                v += np.sum(row[:, :P] ** 2)            # diagonal block (k,k)
                if w > P:
                    v += 2.0 * np.sum(row[:, P:] ** 2)  # blocks l > k
                off += w
        F2[i, j] = v
        F2[j, i] = v

    # use the same rounded X the device matmuls consumed, so the
    # g/r/t terms cancel the diagonal contributions of ||C||^2 exactly
    Xd = X.astype(_np_mm_dtype()).astype(np.float64)
    g = np.einsum("ind,ind->in", Xd, Xd)            # [M, N]
    s = Xd.sum(axis=1)                              # [M, D]
    r = np.einsum("ind,id->in", Xd, s) - g          # [M, N]
    t = r.sum(axis=1)                               # [M]

    S = F2 - g @ g.T - (2.0 / N) * (r @ r.T) + np.outer(t, t) / N ** 2
    S = np.abs(S)
    diag = np.diagonal(S)
    cka = S / np.sqrt(diag[:, None] * diag[None, :] + EPS)
    hsic = cka * np.tril(np.ones((M, M)), k=-1)
    l = np.sum(np.abs(hsic))
    vis = hsic + hsic.T + np.eye(M)
    return vis.astype(np.float32), np.array(l, dtype=np.float32)


def kernel(output):
    X = np.ascontiguousarray(np.asarray(output), dtype=np.float32)
    assert X.shape == (M, N, D)
    res = _run_device(X)
    couts = [r["cout"] for r in res.results]
    return _assemble(X, couts)


# revision 17
# speedup vs baseline: 1.8220x; 1.4479x over previous
"""CKA loss kernel for Trainium2 (8 NeuronCores, SPMD).

Math: for X_i = output[i] (shape [N, D]), the reference builds N x N Gram
matrices K_i = X_i X_i^T (diag zeroed), double-centers them and takes
pairwise inner products. Algebraically this reduces to D x D cross-Grams:

    S[i,j] = <Khat_i, Khat_j>
           = ||X_i^T X_j||_F^2 - g_i.g_j - (2/N) r_i.r_j + t_i t_j / N^2

with g_i[n] = ||X_i[n]||^2, r_i = X_i (X_i^T 1) - g_i, t_i = sum(r_i).
Only the ||X_i^T X_j||_F^2 term is heavy (C_ij = X_i^T X_j, 512x512,
contraction over N=4096) — everything else is O(M N D) and done on host
in float64.

Sharding: the contraction axis N is split across the 8 cores (512 samples
each). Every core computes partial C_ij for all 21 pairs (i<=j) from its
slice — perfectly balanced, no collectives. The host sums the partials in
float64, squares, and assembles the tiny [6,6] output.

The matmuls run in float16 (inputs are ~N(0,1); fp16 rounding of the
inputs perturbs S by ~1e-5 relative — the final cancellation is only ~8x).
Partial C blocks return to the host as float16 as well (entries are
O(100), and the per-entry rounding is ~2^-11 relative, far below the
tolerance). For diagonal pairs C_ii only column-blocks e >= d-block are
computed (block-level symmetry), saving ~12% of the matmul work.
"""

import numpy as np

M, N, D = 6, 4096, 512
NCORES = 8
NLOC = N // NCORES   # samples per core
P = 128              # partitions
NCH = NLOC // P      # contraction chunks per core
KBLK = D // P        # 128-row blocks of C
PAIRS = [(i, j) for i in range(M) for j in range(i, M)]  # 21
EPS = 1e-6

MM_DTYPE = "float16"  # matmul operand/output-staging dtype

_CACHE = {}


def _np_mm_dtype():
    if MM_DTYPE == "float16":
        return np.float16
    import ml_dtypes

    if MM_DTYPE == "bfloat16":
        return np.dtype(ml_dtypes.bfloat16)
    if MM_DTYPE == "float8e4":
        return np.dtype(ml_dtypes.float8_e4m3)
    raise ValueError(MM_DTYPE)


def _build_nc(mm_dtype_name=None):
    mm_dtype_name = mm_dtype_name or MM_DTYPE
    import concourse.bacc as bacc
    import concourse.mybir as mybir
    import concourse.tile as tile

    mm_dt = getattr(mybir.dt, mm_dtype_name)
    fp8 = mm_dtype_name == "float8e4"
    # fp8 runs DoubleRow: 2 contraction sub-rows per partition per matmul
    ks = 2 if fp8 else 1
    nch = NLOC // (P * ks)  # contraction chunks per core
    nc = bacc.Bacc("TRN2", target_bir_lowering=False, debug=False,
                   num_devices=NCORES)
    # chunk-major input: each (layer, chunk) slice is one contiguous
    # region -> large DMA descriptors. Layout [P, ks, D] per chunk.
    x_in = nc.dram_tensor("x", [M, nch, P, ks, D], mm_dt,
                          kind="ExternalInput")
    cout = nc.dram_tensor("cout", [len(PAIRS), P, KBLK * D],
                          mybir.dt.float16, kind="ExternalOutput")

    import concourse.mybir as _mybir
    perf_mode = _mybir.MatmulPerfMode.DoubleRow if fp8 else None

    with tile.TileContext(nc) as tc:
        with (
            tc.tile_pool(name="xin", bufs=1) as xpool,
            tc.tile_pool(name="stage", bufs=3) as spool,
            tc.tile_pool(name="psum", bufs=8, space="PSUM") as ppool,
        ):
            # one tile per (layer, chunk) so matmuls wait only on the
            # chunks they actually read — faster pipeline ramp.  Input
            # DMAs trigger from GpSimd so the Sync queue is free for
            # output DMAs.
            xs = []
            for i in range(M):
                row = []
                for c in range(nch):
                    xt = xpool.tile([P, ks, D], mm_dt, tag=f"x{i}c{c}")
                    nc.gpsimd.dma_start(xt[:], x_in[i, c])
                    row.append(xt)
                xs.append(row)

            ncopy = 0
            for pi, (i, j) in enumerate(PAIRS):
                st = spool.tile([P, KBLK * D], mybir.dt.float16, tag="st")
                off = 0
                for k in range(KBLK):
                    # C_ij rows [128k, 128k+128); for i==j skip column
                    # blocks left of the diagonal block.
                    e0 = k * P if i == j else 0
                    w = D - e0
                    acc = ppool.tile([P, D], mybir.dt.float32, tag="acc")
                    for c in range(nch):
                        nc.tensor.matmul(
                            acc[:, :w],
                            xs[i][c][:, :, k * P:(k + 1) * P],
                            xs[j][c][:, :, e0:],
                            start=(c == 0),
                            stop=(c == nch - 1),
                            perf_mode=perf_mode,
                        )
                    # spread PSUM->SBUF cast-copies over DVE and ACT
                    dst = st[:, off:off + w]
                    if ncopy % 3 == 2:
                        nc.scalar.copy(dst, acc[:, :w])
                    else:
                        nc.vector.tensor_copy(dst, acc[:, :w])
                    ncopy += 1
                    off += w
                nc.sync.dma_start(cout[pi, :, :off], st[:, :off])

    nc.compile()
    return nc


def _get_nc():
    if "nc" not in _CACHE:
        _CACHE["nc"] = _build_nc()
    return _CACHE["nc"]


def _shard_inputs(X):
    """X: [M, N, D] float32 -> per-core low-precision [M, nch, P, ks, D].

    Contraction sample (within a chunk) maps to (ks*128 + p) for fp8
    DoubleRow (ks=2), or just p otherwise (ks=1)."""
    Xh = X.astype(_np_mm_dtype())
    ks = 2 if MM_DTYPE == "float8e4" else 1
    nch = NLOC // (P * ks)
    maps = []
    for c in range(NCORES):
        sl = Xh[:, c * NLOC:(c + 1) * NLOC, :]
        sl = sl.reshape(M, nch, ks, P, D).transpose(0, 1, 3, 2, 4)
        maps.append({"x": np.ascontiguousarray(sl)})
    return maps


def _run_device(X, trace=False):
    from concourse.bass_utils import run_bass_kernel_spmd

    nc = _get_nc()
    res = run_bass_kernel_spmd(nc, _shard_inputs(X),
                               core_ids=list(range(NCORES)), trace=trace)
    return res


def _assemble(X, couts):
    """Host-side float64 assembly of the final CKA outputs.

    couts: per-core [21, P, KBLK*D] arrays of partial C blocks; block k
    of pair (i,j) has width w_k (= D, or D - 128k for diagonal pairs)
    and the blocks are packed back-to-back from column 0.
    """
    Csum = np.zeros((len(PAIRS), P, KBLK * D), np.float64)
    for c in couts:
        Csum += c.astype(np.float64)

    F2 = np.zeros((M, M))
    for pi, (i, j) in enumerate(PAIRS):
        if i != j:
            v = np.sum(Csum[pi] ** 2)
        else:
            v = 0.0
            off = 0
            for k in range(KBLK):
                w = D - k * P
                row = Csum[pi, :, off:off + w]
                v += np.sum(row[:, :P] ** 2)            # diagonal block (k,k)
                if w > P:
                    v += 2.0 * np.sum(row[:, P:] ** 2)  # blocks l > k
                off += w
        F2[i, j] = v
        F2[j, i] = v

    # use the same rounded X the device matmuls consumed, so the
    # g/r/t terms cancel the diagonal contributions of ||C||^2 exactly
    Xd = X.astype(_np_mm_dtype()).astype(np.float64)
    g = np.einsum("ind,ind->in", Xd, Xd)            # [M, N]
    s = Xd.sum(axis=1)                              # [M, D]
    r = np.einsum("ind,id->in", Xd, s) - g          # [M, N]
    t = r.sum(axis=1)                               # [M]

    S = F2 - g @ g.T - (2.0 / N) * (r @ r.T) + np.outer(t, t) / N ** 2
    S = np.abs(S)
    diag = np.diagonal(S)
    cka = S / np.sqrt(diag[:, None] * diag[None, :] + EPS)
    hsic = cka * np.tril(np.ones((M, M)), k=-1)
    l = np.sum(np.abs(hsic))
    vis = hsic + hsic.T + np.eye(M)
    return vis.astype(np.float32), np.array(l, dtype=np.float32)


def kernel(output):
    X = np.ascontiguousarray(np.asarray(output), dtype=np.float32)
    assert X.shape == (M, N, D)
    res = _run_device(X)
    couts = [r["cout"] for r in res.results]
    return _assemble(X, couts)
